# revision 23
# baseline (speedup 1.0000x reference)
"""Multi-head attention (B=4, S=2048, D=1024, H=16, Dh=64) on 8 TRN2 NeuronCores.

Sharding: core = (batch, head_group) with 4 batches x 2 head-groups of 8 heads.
Fully data-parallel SPMD - no collectives. Each core computes
out[b, :, hg*512:(hg+1)*512].

Per-core kernel (bf16 matmuls, fp32 PSUM accumulation):
  host pre-casts seq inputs + weights to bf16 and transposes seq inputs to
  [d_model, seq].  The attention kt-loop is software-pipelined one kt ahead
  (scores for kt+1 issue as soon as the exp of kt frees the PSUM slot) so the
  exp stream runs back-to-back; quad prologues are hoisted into the previous
  quad's tail.  Projection work (K/V/Q) is broken into 8-matmul units and
  injected one-per-PSUM-pass into the early quad-loops' iterations.  A
  fraction of the exps runs on VectorE via a Schraudolph bf16-exp (one
  fused multiply-add + round straight into the bf16 bit pattern).
  Denominators via ones-matmuls col-tiled 4-way; O^T and the sums are
  PE-transposed back to q-major (bf16) and normalized on VectorE.
"""

from contextlib import ExitStack

import numpy as np
import ml_dtypes

import concourse.bass as bass
import concourse.bacc as bacc
import concourse.mybir as mybir
import concourse.tile as tile
from concourse.bass_utils import run_bass_kernel_spmd
from concourse.masks import make_identity

B = 4
SEQ = 2048
DM = 1024
H = 16
DH = 64
NCORES = 8
CPC = 512          # output columns per core (8 heads x 64)
P = 128
NQB = SEQ // 512   # q blocks of 512
NKT = SEQ // P     # k tiles of 128
NDT = DM // P      # d_model tiles of 128

F32 = mybir.dt.float32
BF16 = mybir.dt.bfloat16
I16 = mybir.dt.int16
NPBF16 = ml_dtypes.bfloat16
EXP = mybir.ActivationFunctionType.Exp
MULT = None  # set below

SCHR_A = 0.125 * 128.0 / float(np.log(2.0))   # fold the 1/sqrt(dh) scale in
SCHR_B = 16249.2                              # 127*128 - C (C tuned ~ min-RMS)

_compiled = None


def _emit(ctx: ExitStack, tc: tile.TileContext, qt, kt, vt, wq, wk, wv, bmask, out, out_s, seq=SEQ):
    nc = tc.nc
    NKT = seq // P
    MULT = mybir.AluOpType.mult
    ADD = mybir.AluOpType.add

    proj = ctx.enter_context(tc.tile_pool(name="proj", bufs=1))
    small = ctx.enter_context(tc.tile_pool(name="small", bufs=1))
    stage = ctx.enter_context(tc.tile_pool(name="stage", bufs=5))
    wpool = ctx.enter_context(tc.tile_pool(name="wpool", bufs=1))
    epool = ctx.enter_context(tc.tile_pool(name="epool", bufs=8))
    opool = ctx.enter_context(tc.tile_pool(name="opool", bufs=2))
    oparts = ctx.enter_context(tc.tile_pool(name="oparts", bufs=2))
    ps_sc = ctx.enter_context(tc.tile_pool(name="ps_sc", bufs=2, space="PSUM"))
    ps_ot = ctx.enter_context(tc.tile_pool(name="ps_ot", bufs=2, space="PSUM"))
    ps_sm = ctx.enter_context(tc.tile_pool(name="ps_sm", bufs=1, space="PSUM"))
    ps_tr = ctx.enter_context(tc.tile_pool(name="ps_tr", bufs=1, space="PSUM"))

    kt_r = kt.ap().rearrange("(dt p) q -> p dt q", p=P)
    vt_r = vt.ap().rearrange("(dt p) q -> p dt q", p=P)
    qt_r = qt.ap().rearrange("(dt p) q -> p dt q", p=P)
    srcs = {"k": kt_r, "v": vt_r, "q": qt_r}
    stage_tiles = {}

    def stage_blk(kind, blk):
        st = stage.tile([P, NDT, 512], BF16, tag="stage", name=f"st_{kind}{blk}")
        nc.sync.dma_start(st[:], srcs[kind][:, :, blk * 512:(blk + 1) * 512])
        stage_tiles[(kind, blk)] = st

    # ---- startup: critical-path DMAs first (k0, wk, q0, wq) ------------
    stage_blk("k", 0)
    w_sb = {}

    def w_dma(name, w):
        w_sb[name] = wpool.tile([P, NDT, CPC], BF16, tag=name, name=name)
        nc.sync.dma_start(w_sb[name][:], w.ap().rearrange("(dt p) c -> p dt c", p=P))

    bmask_sb = small.tile([P, NKT], F32)
    nc.sync.dma_start(bmask_sb[:], bmask.ap())
    w_dma("wk", wk)
    stage_blk("q", 0)
    w_dma("wq", wq)

    ident = small.tile([P, P], BF16)
    make_identity(nc, ident[:])
    ones = small.tile([P, 1], BF16)
    nc.vector.memset(ones[:], 1.0)
    # Schraudolph per-partition bias: B + mask * A' (saturates masked lanes)
    bschr_sb = small.tile([P, NKT], F32)
    nc.vector.tensor_scalar(bschr_sb[:], bmask_sb[:], 128.0 / float(np.log(2.0)),
                            SCHR_B, MULT, ADD)

    kproj = [proj.tile([P, seq], BF16, tag=f"kproj{p}", name=f"kproj{p}") for p in range(4)]
    qproj = [proj.tile([P, seq], BF16, tag=f"qproj{p}", name=f"qproj{p}") for p in range(4)]
    v_sb = proj.tile([P, NKT, 512], BF16, tag="v_sb")

    wname = {"k": "wk", "q": "wq", "v": "wv"}
    pdsts = {"k": kproj, "q": qproj}

    def unit_mms(acc, kind, blk, idx):
        """8 accumulating matmuls + 1 copy for one projection unit."""
        st = stage_tiles[(kind, blk)]
        for dt in range(NDT):
            if kind == "v":
                nc.tensor.matmul(
                    acc, st[:, dt, 128 * idx:128 * (idx + 1)], w_sb["wv"][:, dt, :],
                    start=(dt == 0), stop=(dt == NDT - 1),
                )
            else:
                nc.tensor.matmul(
                    acc, w_sb[wname[kind]][:, dt, 128 * idx:128 * (idx + 1)], st[:, dt, :],
                    start=(dt == 0), stop=(dt == NDT - 1),
                )
        if kind == "v":
            nc.vector.tensor_copy(v_sb[:, blk * 4 + idx, :], acc)
        else:
            nc.vector.tensor_copy(pdsts[kind][idx][:, blk * 512:(blk + 1) * 512], acc)

    def emit_units(units):
        for u in units:
            if u[0] == "stage":
                stage_blk(u[1], u[2])
            else:
                ps = ps_sc.tile([P, 1024], F32, tag="scores")
                unit_mms(ps[:, 0:512], *u)

    # ---- phase A ---------------------------------------------------------
    emit_units([("k", 0, 0), ("k", 0, 1),
                ("q", 0, 0), ("q", 0, 1),
                ("stage", "v", 0)])
    w_dma("wv", wv)
    stage_blk("k", 1)

    # per-loop injection schedules: {loop: {kt: [unit...]}}
    inject = {
        0: {0: [("v", 0, 0), ("k", 1, 0)], 1: [("v", 0, 1), ("k", 1, 1)],
            2: [("stage", "v", 1), ("v", 0, 2)], 3: [("v", 0, 3)],
            4: [("stage", "k", 2), ("v", 1, 0)], 5: [("v", 1, 1), ("k", 2, 0)],
            6: [("v", 1, 2), ("k", 2, 1)], 7: [("v", 1, 3)],
            8: [("stage", "v", 2), ("stage", "k", 3), ("v", 2, 0)],
            9: [("v", 2, 1), ("k", 3, 0)], 10: [("v", 2, 2), ("k", 3, 1)],
            11: [("v", 2, 3)],
            12: [("stage", "v", 3), ("stage", "q", 1), ("v", 3, 0)],
            13: [("v", 3, 1), ("q", 1, 0)], 14: [("v", 3, 2), ("q", 1, 1)],
            15: [("v", 3, 3)]},
        1: {2: [("stage", "k", 0)], 3: [("k", 0, 2)], 5: [("k", 0, 3)],
            7: [("stage", "k", 1), ("k", 1, 2)], 9: [("k", 1, 3)],
            11: [("stage", "q", 2), ("q", 2, 0)], 13: [("q", 2, 1)]},
        2: {2: [("stage", "k", 2)], 3: [("k", 2, 2)], 5: [("k", 2, 3)],
            7: [("stage", "k", 3), ("k", 3, 2)], 9: [("k", 3, 3)],
            11: [("stage", "q", 3), ("q", 3, 0)], 13: [("q", 3, 1)]},
        3: {2: [("stage", "q", 0)], 3: [("q", 0, 2)], 5: [("q", 0, 3)],
            7: [("stage", "q", 1), ("q", 1, 2)], 9: [("q", 1, 3)]},
        4: {3: [("stage", "q", 2), ("q", 2, 2)], 5: [("q", 2, 3)]},
        5: {3: [("stage", "q", 3), ("q", 3, 2)], 5: [("q", 3, 3)]},
    }
    CHUNK_KTS = (8, 10, 12, 14)

    loops = [(qb, quad) for quad in range(2) for qb in range(NQB)]

    def scores_pair(li, kt_i, pi):
        qb, quad = loops[li]
        pr = 2 * quad + pi
        st_ps = ps_sc.tile([P, 1024], F32, tag="scores")
        for hh in range(2):
            rows = slice(64 * hh, 64 * (hh + 1))
            nc.tensor.matmul(
                st_ps[:, 512 * hh:512 * (hh + 1)],
                kproj[pr][rows, kt_i * P:(kt_i + 1) * P],
                qproj[pr][rows, qb * 512 + 0:qb * 512 + 512],
                start=True,
                stop=True,
                tile_position=(64 * hh, 0),
            )
        return st_ps

    carry = None
    pending = []
    for li, (qb, quad) in enumerate(loops):
        inj = inject.get(li, {})
        ot_ps = [ps_ot.tile([P, 512], F32, tag="ot", name=f"ot{i}") for i in range(2)]
        sm_ps = ps_sm.tile([P, 512], F32, tag="sums")

        def av(kt_i, pi, e):
            pr = 2 * quad + pi
            for hh in range(2):
                cols = slice(128 * pr + 64 * hh, 128 * pr + 64 * (hh + 1))
                nc.tensor.matmul(
                    ot_ps[pi][64 * hh:64 * (hh + 1), :],
                    v_sb[:, kt_i, cols],
                    e[:, 512 * hh:512 * (hh + 1)],
                    start=(kt_i == 0),
                    stop=(kt_i == NKT - 1),
                    tile_position=(0, 64 * hh),
                    skip_group_check=(hh == 1),
                )

        cur = carry if carry is not None else [scores_pair(li, 0, 0), scores_pair(li, 0, 1)]
        for kt_i in range(NKT):
            emit_units(inj.get(kt_i, []))
            if pending and kt_i in CHUNK_KTS:
                pending[CHUNK_KTS.index(kt_i)]()
            nxt = [None, None]
            e_tiles = []
            for pi in range(2):
                e = epool.tile([P, 1024], BF16, tag="e")
                if li > 0 and ((pi == 1 and kt_i % 2 == 1)
                               or (pi == 0 and li >= 3 and kt_i % 8 == 2)):
                    nc.vector.tensor_scalar(
                        e[:].bitcast(I16), cur[pi][:],
                        SCHR_A, bschr_sb[:, kt_i:kt_i + 1], MULT, ADD,
                    )
                else:
                    nc.scalar.activation(
                        e[:], cur[pi][:], EXP,
                        bias=bmask_sb[:, kt_i:kt_i + 1], scale=0.125,
                    )
                e_tiles.append(e)
                if kt_i + 1 < NKT:
                    nxt[pi] = scores_pair(li, kt_i + 1, pi)
                av(kt_i, pi, e)
            for j in range(4):
                nc.tensor.matmul(
                    sm_ps[32 * j:32 * j + 1, :],
                    ones[:],
                    e_tiles[j // 2][:, 512 * (j % 2):512 * (j % 2 + 1)],
                    start=(kt_i == 0),
                    stop=(kt_i == NKT - 1),
                    tile_position=(0, 32 * j),
                    skip_group_check=(j > 0),
                )
            cur = nxt

        # ---- early tail: free the PSUM accumulators -------------------
        sums_sb = opool.tile([P, 512], BF16, tag="sums_sb")
        for j in range(4):
            nc.vector.tensor_copy(
                sums_sb[32 * j:32 * j + 1, :], sm_ps[32 * j:32 * j + 1, :]
            )
        nc.sync.dma_start(out_s.ap()[quad, qb], sums_sb[0:128:32, :])
        ot_sb = []
        for pi in range(2):
            t = opool.tile([P, 512], BF16, tag=f"ot_sb{pi}", name=f"ot_sb{pi}")
            nc.vector.tensor_copy(t[:], ot_ps[pi][:])
            ot_sb.append(t)

        # ---- hoist the next loop's prologue ---------------------------
        if li + 1 < len(loops):
            carry = [scores_pair(li + 1, 0, 0), scores_pair(li + 1, 0, 1)]
        else:
            carry = None

        # ---- late tail: chunked, deferred into the next loop ----------
        def make_chunks(li, qb, quad, ot_sb):
            cell = {}
            last = li == len(loops) - 1

            def c_chunk(c):
                if "o_part" not in cell:
                    cell["o_part"] = oparts.tile([P, 4, 256], F32, tag="opart", name="opart")
                o_part = cell["o_part"]
                for pi in range(2):
                    tr_o = ps_tr.tile([P, P], BF16, tag="trp")
                    nc.tensor.transpose(tr_o[:], ot_sb[pi][:, c * P:(c + 1) * P], ident[:])
                    nc.vector.tensor_copy(
                        o_part[:, c, 128 * pi:128 * (pi + 1)], tr_o[:]
                    )
                rows = out.ap()[qb * 512 + c * P:qb * 512 + (c + 1) * P, :]
                if last:
                    for h in range(2):
                        nc.sync.dma_start(
                            rows[:, quad * 256 + h * 128:quad * 256 + (h + 1) * 128],
                            o_part[:, c, h * 128:(h + 1) * 128],
                        )
                else:
                    nc.sync.dma_start(
                        rows[:, quad * 256:(quad + 1) * 256], o_part[:, c, :]
                    )

            return [lambda c=c: c_chunk(c) for c in range(4)]

        chunks = make_chunks(li, qb, quad, ot_sb)
        if li == len(loops) - 1:
            for ch in chunks:
                ch()
            pending = []
        else:
            pending = chunks


def build(seq=SEQ):
    global _compiled
    if seq == SEQ and _compiled is not None:
        return _compiled
    nc = bacc.Bacc("TRN2", target_bir_lowering=False, debug=False)
    qt = nc.dram_tensor("qt", [DM, seq], BF16, kind="ExternalInput")
    kt = nc.dram_tensor("kt", [DM, seq], BF16, kind="ExternalInput")
    vt = nc.dram_tensor("vt", [DM, seq], BF16, kind="ExternalInput")
    wq = nc.dram_tensor("wq", [DM, CPC], BF16, kind="ExternalInput")
    wk = nc.dram_tensor("wk", [DM, CPC], BF16, kind="ExternalInput")
    wv = nc.dram_tensor("wv", [DM, CPC], BF16, kind="ExternalInput")
    bmask = nc.dram_tensor("bmask", [P, seq // P], F32, kind="ExternalInput")
    out = nc.dram_tensor("out", [seq, CPC], F32, kind="ExternalOutput")
    out_s = nc.dram_tensor("sums", [2, seq // 512, 4, 512], BF16, kind="ExternalOutput")
    with tile.TileContext(nc) as tc:
        with ExitStack() as ctx:
            _emit(ctx, tc, qt, kt, vt, wq, wk, wv, bmask, out, out_s, seq=seq)
    nc.compile()
    if seq == SEQ:
        _compiled = nc
    return nc


def make_in_maps(Q_seq, K_seq, V_seq, V_len, WQ, WK, WV):
    in_maps = []
    wq16 = np.asarray(WQ, np.float32).astype(NPBF16)
    wk16 = np.asarray(WK, np.float32).astype(NPBF16)
    wv16 = np.asarray(WV, np.float32).astype(NPBF16)
    for core in range(NCORES):
        b, hg = divmod(core, 2)
        cols = slice(hg * CPC, (hg + 1) * CPC)
        bm = np.zeros((P, NKT), np.float32)
        vl = int(V_len[b, 0])
        bm[vl % P, vl // P] = -1e6
        in_maps.append(
            {
                "qt": np.ascontiguousarray(np.asarray(Q_seq[b], np.float32).astype(NPBF16).T),
                "kt": np.ascontiguousarray(np.asarray(K_seq[b], np.float32).astype(NPBF16).T),
                "vt": np.ascontiguousarray(np.asarray(V_seq[b], np.float32).astype(NPBF16).T),
                "wq": np.ascontiguousarray(wq16[:, cols]),
                "wk": np.ascontiguousarray(wk16[:, cols]),
                "wv": np.ascontiguousarray(wv16[:, cols]),
                "bmask": bm,
            }
        )
    return in_maps


def kernel(Q_seq, K_seq, V_seq, Q_len, V_len, WQ, WK, WV, _trace=False):
    nc = build()
    in_maps = make_in_maps(Q_seq, K_seq, V_seq, V_len, WQ, WK, WV)
    res = run_bass_kernel_spmd(
        nc, in_maps, core_ids=list(range(NCORES)), trace=_trace
    )
    out = np.empty((B, SEQ, H * DH), np.float32)
    for core in range(NCORES):
        b, hg = divmod(core, 2)
        o = np.array(res.results[core]["out"], np.float32)
        s = np.asarray(res.results[core]["sums"], np.float32)  # [2, NQB, 4, 512]
        for quad in range(2):
            for qb in range(SEQ // 512):
                for j in range(4):
                    cols = 128 * (2 * quad + j // 2) + 64 * (j % 2)
                    o[qb * 512:(qb + 1) * 512, cols:cols + 64] /= \
                        s[quad, qb, j][:, None]
        out[b, :, hg * CPC:(hg + 1) * CPC] = o
    for b in range(B):
        out[b, int(Q_len[b, 0]), :] = 0.0
    if _trace:
        kernel._last_results = res
    return out


# revision 24
# speedup vs baseline: 1.1359x; 1.1359x over previous
"""Multi-head attention (B=4, S=2048, D=1024, H=16, Dh=64) on 8 TRN2 NeuronCores.

Sharding: core = (batch, head_group) with 4 batches x 2 head-groups of 8 heads.
Fully data-parallel SPMD - no collectives. Each core computes
out[b, :, hg*512:(hg+1)*512].

Per-core kernel (bf16 matmuls, fp32 PSUM accumulation):
  host pre-casts seq inputs + weights to bf16 and transposes seq inputs to
  [d_model, seq].  The attention kt-loop is software-pipelined one kt ahead
  (scores for kt+1 issue as soon as the exp of kt frees the PSUM slot) so the
  exp stream runs back-to-back; quad prologues are hoisted into the previous
  quad's tail.  Projection work (K/V/Q) is broken into 8-matmul units and
  injected one-per-PSUM-pass into the early quad-loops' iterations.  A
  fraction of the exps runs on VectorE via a Schraudolph bf16-exp (one
  fused multiply-add + round straight into the bf16 bit pattern).
  Denominators via ones-matmuls col-tiled 4-way; O^T and the sums are
  PE-transposed back to q-major (bf16) and normalized on VectorE.
"""

from contextlib import ExitStack

import numpy as np
import ml_dtypes

import concourse.bass as bass
import concourse.bacc as bacc
import concourse.mybir as mybir
import concourse.tile as tile
from concourse.bass_utils import run_bass_kernel_spmd
from concourse.masks import make_identity

B = 4
SEQ = 2048
DM = 1024
H = 16
DH = 64
NCORES = 8
CPC = 512          # output columns per core (8 heads x 64)
P = 128
NQB = SEQ // 512   # q blocks of 512
NKT = SEQ // P     # k tiles of 128
NDT = DM // P      # d_model tiles of 128

F32 = mybir.dt.float32
BF16 = mybir.dt.bfloat16
I16 = mybir.dt.int16
NPBF16 = ml_dtypes.bfloat16
EXP = mybir.ActivationFunctionType.Exp
MULT = None  # set below

SCHR_A = 0.125 * 128.0 / float(np.log(2.0))   # fold the 1/sqrt(dh) scale in
SCHR_B = 16249.2                              # 127*128 - C (C tuned ~ min-RMS)

_compiled = None


def _emit(ctx: ExitStack, tc: tile.TileContext, qt, kt, vt, wq, wk, wv, bmask, out, seq=SEQ):
    nc = tc.nc
    NKT = seq // P
    MULT = mybir.AluOpType.mult
    ADD = mybir.AluOpType.add

    proj = ctx.enter_context(tc.tile_pool(name="proj", bufs=1))
    small = ctx.enter_context(tc.tile_pool(name="small", bufs=1))
    stage = ctx.enter_context(tc.tile_pool(name="stage", bufs=5))
    wpool = ctx.enter_context(tc.tile_pool(name="wpool", bufs=1))
    epool = ctx.enter_context(tc.tile_pool(name="epool", bufs=8))
    opool = ctx.enter_context(tc.tile_pool(name="opool", bufs=2))
    oparts = ctx.enter_context(tc.tile_pool(name="oparts", bufs=2))
    ps_sc = ctx.enter_context(tc.tile_pool(name="ps_sc", bufs=2, space="PSUM"))
    ps_ot = ctx.enter_context(tc.tile_pool(name="ps_ot", bufs=2, space="PSUM"))
    ps_sm = ctx.enter_context(tc.tile_pool(name="ps_sm", bufs=1, space="PSUM"))
    ps_tr = ctx.enter_context(tc.tile_pool(name="ps_tr", bufs=1, space="PSUM"))

    kt_r = kt.ap().rearrange("(dt p) q -> p dt q", p=P)
    vt_r = vt.ap().rearrange("(dt p) q -> p dt q", p=P)
    qt_r = qt.ap().rearrange("(dt p) q -> p dt q", p=P)
    srcs = {"k": kt_r, "v": vt_r, "q": qt_r}
    stage_tiles = {}

    def stage_blk(kind, blk):
        st = stage.tile([P, NDT, 512], BF16, tag="stage", name=f"st_{kind}{blk}")
        nc.sync.dma_start(st[:], srcs[kind][:, :, blk * 512:(blk + 1) * 512])
        stage_tiles[(kind, blk)] = st

    # ---- startup: critical-path DMAs first (k0, wk, q0, wq) ------------
    stage_blk("k", 0)
    w_sb = {}

    def w_dma(name, w):
        w_sb[name] = wpool.tile([P, NDT, CPC], BF16, tag=name, name=name)
        nc.sync.dma_start(w_sb[name][:], w.ap().rearrange("(dt p) c -> p dt c", p=P))

    bmask_sb = small.tile([P, NKT], F32)
    nc.sync.dma_start(bmask_sb[:], bmask.ap())
    w_dma("wk", wk)
    stage_blk("q", 0)
    w_dma("wq", wq)

    ident = small.tile([P, P], BF16)
    make_identity(nc, ident[:])
    ones = small.tile([P, 1], BF16)
    nc.vector.memset(ones[:], 1.0)
    # Schraudolph per-partition bias: B + mask * A' (saturates masked lanes)
    bschr_sb = small.tile([P, NKT], F32)
    nc.vector.tensor_scalar(bschr_sb[:], bmask_sb[:], 128.0 / float(np.log(2.0)),
                            SCHR_B, MULT, ADD)

    kproj = [proj.tile([P, seq], BF16, tag=f"kproj{p}", name=f"kproj{p}") for p in range(4)]
    qproj = [proj.tile([P, seq], BF16, tag=f"qproj{p}", name=f"qproj{p}") for p in range(4)]
    v_sb = proj.tile([P, NKT, 512], BF16, tag="v_sb")

    wname = {"k": "wk", "q": "wq", "v": "wv"}
    pdsts = {"k": kproj, "q": qproj}

    def unit_mms(acc, kind, blk, idx):
        """8 accumulating matmuls + 1 copy for one projection unit."""
        st = stage_tiles[(kind, blk)]
        for dt in range(NDT):
            if kind == "v":
                nc.tensor.matmul(
                    acc, st[:, dt, 128 * idx:128 * (idx + 1)], w_sb["wv"][:, dt, :],
                    start=(dt == 0), stop=(dt == NDT - 1),
                )
            else:
                nc.tensor.matmul(
                    acc, w_sb[wname[kind]][:, dt, 128 * idx:128 * (idx + 1)], st[:, dt, :],
                    start=(dt == 0), stop=(dt == NDT - 1),
                )
        if kind == "v":
            nc.vector.tensor_copy(v_sb[:, blk * 4 + idx, :], acc)
        else:
            nc.vector.tensor_copy(pdsts[kind][idx][:, blk * 512:(blk + 1) * 512], acc)

    def emit_units(units):
        for u in units:
            if u[0] == "stage":
                stage_blk(u[1], u[2])
            else:
                ps = ps_sc.tile([P, 1024], F32, tag="scores")
                unit_mms(ps[:, 0:512], *u)

    # ---- phase A ---------------------------------------------------------
    emit_units([("k", 0, 0), ("k", 0, 1),
                ("q", 0, 0), ("q", 0, 1),
                ("stage", "v", 0)])
    w_dma("wv", wv)
    stage_blk("k", 1)

    # per-loop injection schedules: {loop: {kt: [unit...]}}
    inject = {
        0: {0: [("v", 0, 0), ("k", 1, 0)], 1: [("v", 0, 1), ("k", 1, 1)],
            2: [("stage", "v", 1), ("v", 0, 2)], 3: [("v", 0, 3)],
            4: [("stage", "k", 2), ("v", 1, 0)], 5: [("v", 1, 1), ("k", 2, 0)],
            6: [("v", 1, 2), ("k", 2, 1)], 7: [("v", 1, 3)],
            8: [("stage", "v", 2), ("stage", "k", 3), ("v", 2, 0)],
            9: [("v", 2, 1), ("k", 3, 0)], 10: [("v", 2, 2), ("k", 3, 1)],
            11: [("v", 2, 3)],
            12: [("stage", "v", 3), ("stage", "q", 1), ("v", 3, 0)],
            13: [("v", 3, 1), ("q", 1, 0)], 14: [("v", 3, 2), ("q", 1, 1)],
            15: [("v", 3, 3)]},
        1: {2: [("stage", "k", 0)], 3: [("k", 0, 2)], 5: [("k", 0, 3)],
            7: [("stage", "k", 1), ("k", 1, 2)], 9: [("k", 1, 3)],
            11: [("stage", "q", 2), ("q", 2, 0)], 13: [("q", 2, 1)]},
        2: {2: [("stage", "k", 2)], 3: [("k", 2, 2)], 5: [("k", 2, 3)],
            7: [("stage", "k", 3), ("k", 3, 2)], 9: [("k", 3, 3)],
            11: [("stage", "q", 3), ("q", 3, 0)], 13: [("q", 3, 1)]},
        3: {2: [("stage", "q", 0)], 3: [("q", 0, 2)], 5: [("q", 0, 3)],
            7: [("stage", "q", 1), ("q", 1, 2)], 9: [("q", 1, 3)]},
        4: {3: [("stage", "q", 2), ("q", 2, 2)], 5: [("q", 2, 3)]},
        5: {3: [("stage", "q", 3), ("q", 3, 2)], 5: [("q", 3, 3)]},
    }
    CHUNK_KTS = (6, 8, 10, 12, 14)

    loops = [(qb, quad) for quad in range(2) for qb in range(NQB)]

    def scores_pair(li, kt_i, pi):
        qb, quad = loops[li]
        pr = 2 * quad + pi
        st_ps = ps_sc.tile([P, 1024], F32, tag="scores")
        for hh in range(2):
            rows = slice(64 * hh, 64 * (hh + 1))
            nc.tensor.matmul(
                st_ps[:, 512 * hh:512 * (hh + 1)],
                kproj[pr][rows, kt_i * P:(kt_i + 1) * P],
                qproj[pr][rows, qb * 512 + 0:qb * 512 + 512],
                start=True,
                stop=True,
                tile_position=(64 * hh, 0),
            )
        return st_ps

    carry = None
    pending = []
    for li, (qb, quad) in enumerate(loops):
        inj = inject.get(li, {})
        ot_ps = [ps_ot.tile([P, 512], F32, tag="ot", name=f"ot{i}") for i in range(2)]
        sm_ps = ps_sm.tile([P, 512], F32, tag="sums")

        def av(kt_i, pi, e):
            pr = 2 * quad + pi
            for hh in range(2):
                cols = slice(128 * pr + 64 * hh, 128 * pr + 64 * (hh + 1))
                nc.tensor.matmul(
                    ot_ps[pi][64 * hh:64 * (hh + 1), :],
                    v_sb[:, kt_i, cols],
                    e[:, 512 * hh:512 * (hh + 1)],
                    start=(kt_i == 0),
                    stop=(kt_i == NKT - 1),
                    tile_position=(0, 64 * hh),
                    skip_group_check=(hh == 1),
                )

        cur = carry if carry is not None else [scores_pair(li, 0, 0), scores_pair(li, 0, 1)]
        for kt_i in range(NKT):
            emit_units(inj.get(kt_i, []))
            if pending and kt_i in CHUNK_KTS:
                pending[CHUNK_KTS.index(kt_i)]()
            nxt = [None, None]
            e_tiles = []
            for pi in range(2):
                e = epool.tile([P, 1024], BF16, tag="e")
                if li > 0 and pi == 1 and kt_i % 2 == 1:
                    nc.vector.tensor_scalar(
                        e[:].bitcast(I16), cur[pi][:],
                        SCHR_A, bschr_sb[:, kt_i:kt_i + 1], MULT, ADD,
                    )
                else:
                    nc.scalar.activation(
                        e[:], cur[pi][:], EXP,
                        bias=bmask_sb[:, kt_i:kt_i + 1], scale=0.125,
                    )
                e_tiles.append(e)
                if kt_i + 1 < NKT:
                    nxt[pi] = scores_pair(li, kt_i + 1, pi)
                av(kt_i, pi, e)
            for j in range(4):
                nc.tensor.matmul(
                    sm_ps[32 * j:32 * j + 1, :],
                    ones[:],
                    e_tiles[j // 2][:, 512 * (j % 2):512 * (j % 2 + 1)],
                    start=(kt_i == 0),
                    stop=(kt_i == NKT - 1),
                    tile_position=(0, 32 * j),
                    skip_group_check=(j > 0),
                )
            cur = nxt

        # ---- early tail: free the PSUM accumulators -------------------
        sums_sb = opool.tile([P, 512], BF16, tag="sums_sb")
        for j in range(4):
            nc.vector.tensor_copy(
                sums_sb[32 * j:32 * j + 1, :], sm_ps[32 * j:32 * j + 1, :]
            )
        ot_sb = []
        for pi in range(2):
            t = opool.tile([P, 512], BF16, tag=f"ot_sb{pi}", name=f"ot_sb{pi}")
            nc.vector.tensor_copy(t[:], ot_ps[pi][:])
            ot_sb.append(t)

        # ---- hoist the next loop's prologue ---------------------------
        if li + 1 < len(loops):
            carry = [scores_pair(li + 1, 0, 0), scores_pair(li + 1, 0, 1)]
        else:
            carry = None

        # ---- late tail: chunked, deferred into the next loop ----------
        def make_chunks(li, qb, quad, sums_sb, ot_sb):
            cell = {}
            last = li == len(loops) - 1

            def rcp_chunk():
                rcp = opool.tile([P, 16], F32, tag="rcp")
                for c in range(4):
                    tr_s = ps_tr.tile([P, P], BF16, tag="trp")
                    nc.tensor.transpose(tr_s[:], sums_sb[:, c * P:(c + 1) * P], ident[:])
                    for j in range(4):
                        nc.vector.reciprocal(
                            rcp[:, 4 * c + j:4 * c + j + 1], tr_s[:, 32 * j:32 * j + 1]
                        )
                cell["rcp"] = rcp
                cell["o_part"] = oparts.tile([P, 4, 256], F32, tag="opart", name="opart")

            def c_chunk(c):
                rcp, o_part = cell["rcp"], cell["o_part"]
                for pi in range(2):
                    tr_o = ps_tr.tile([P, P], BF16, tag="trp")
                    nc.tensor.transpose(tr_o[:], ot_sb[pi][:, c * P:(c + 1) * P], ident[:])
                    for hh in range(2):
                        lh = 2 * pi + hh
                        nc.vector.tensor_scalar(
                            o_part[:, c, 64 * lh:64 * (lh + 1)],
                            tr_o[:, 64 * hh:64 * (hh + 1)],
                            rcp[:, 4 * c + lh:4 * c + lh + 1],
                            None,
                            MULT,
                        )
                rows = out.ap()[qb * 512 + c * P:qb * 512 + (c + 1) * P, :]
                if last:
                    for h in range(2):
                        nc.sync.dma_start(
                            rows[:, quad * 256 + h * 128:quad * 256 + (h + 1) * 128],
                            o_part[:, c, h * 128:(h + 1) * 128],
                        )
                else:
                    nc.sync.dma_start(
                        rows[:, quad * 256:(quad + 1) * 256], o_part[:, c, :]
                    )

            return [rcp_chunk] + [lambda c=c: c_chunk(c) for c in range(4)]

        chunks = make_chunks(li, qb, quad, sums_sb, ot_sb)
        if li == len(loops) - 1:
            for ch in chunks:
                ch()
            pending = []
        else:
            pending = chunks


def build(seq=SEQ):
    global _compiled
    if seq == SEQ and _compiled is not None:
        return _compiled
    nc = bacc.Bacc("TRN2", target_bir_lowering=False, debug=False)
    qt = nc.dram_tensor("qt", [DM, seq], BF16, kind="ExternalInput")
    kt = nc.dram_tensor("kt", [DM, seq], BF16, kind="ExternalInput")
    vt = nc.dram_tensor("vt", [DM, seq], BF16, kind="ExternalInput")
    wq = nc.dram_tensor("wq", [DM, CPC], BF16, kind="ExternalInput")
    wk = nc.dram_tensor("wk", [DM, CPC], BF16, kind="ExternalInput")
    wv = nc.dram_tensor("wv", [DM, CPC], BF16, kind="ExternalInput")
    bmask = nc.dram_tensor("bmask", [P, seq // P], F32, kind="ExternalInput")
    out = nc.dram_tensor("out", [seq, CPC], F32, kind="ExternalOutput")
    with tile.TileContext(nc) as tc:
        with ExitStack() as ctx:
            _emit(ctx, tc, qt, kt, vt, wq, wk, wv, bmask, out, seq=seq)
    nc.compile()
    if seq == SEQ:
        _compiled = nc
    return nc


def make_in_maps(Q_seq, K_seq, V_seq, V_len, WQ, WK, WV):
    in_maps = []
    wq16 = np.asarray(WQ, np.float32).astype(NPBF16)
    wk16 = np.asarray(WK, np.float32).astype(NPBF16)
    wv16 = np.asarray(WV, np.float32).astype(NPBF16)
    for core in range(NCORES):
        b, hg = divmod(core, 2)
        cols = slice(hg * CPC, (hg + 1) * CPC)
        bm = np.zeros((P, NKT), np.float32)
        vl = int(V_len[b, 0])
        bm[vl % P, vl // P] = -1e6
        in_maps.append(
            {
                "qt": np.ascontiguousarray(np.asarray(Q_seq[b], np.float32).astype(NPBF16).T),
                "kt": np.ascontiguousarray(np.asarray(K_seq[b], np.float32).astype(NPBF16).T),
                "vt": np.ascontiguousarray(np.asarray(V_seq[b], np.float32).astype(NPBF16).T),
                "wq": np.ascontiguousarray(wq16[:, cols]),
                "wk": np.ascontiguousarray(wk16[:, cols]),
                "wv": np.ascontiguousarray(wv16[:, cols]),
                "bmask": bm,
            }
        )
    return in_maps


def kernel(Q_seq, K_seq, V_seq, Q_len, V_len, WQ, WK, WV, _trace=False):
    nc = build()
    in_maps = make_in_maps(Q_seq, K_seq, V_seq, V_len, WQ, WK, WV)
    res = run_bass_kernel_spmd(
        nc, in_maps, core_ids=list(range(NCORES)), trace=_trace
    )
    out = np.empty((B, SEQ, H * DH), np.float32)
    for core in range(NCORES):
        b, hg = divmod(core, 2)
        out[b, :, hg * CPC:(hg + 1) * CPC] = res.results[core]["out"]
    for b in range(B):
        out[b, int(Q_len[b, 0]), :] = 0.0
    if _trace:
        kernel._last_results = res
    return out
